# revision 9
# baseline (speedup 1.0000x reference)
"""Causal self-attention (GQA + partial RoPE + q_gain) Trainium2 Bass kernel. v2

Model: B=4, T=2048, D=2048, H=16 q-heads, Hkv=4 kv-heads, hD=128, ROPE=64.
Sharding: 8 cores = 4 batches x 2 head-halves (heads hf*8..hf*8+7, kv heads 2hf, 2hf+1).
Wq/Wkv column-sharded, Wo row-sharded; host sums the two partial outputs per batch.

v2 changes vs v1 (716.9us):
- Stage-interleaved: for each 512-token block tb, projections A(tb) then
  attention B(:, tb) for all heads with queries in block tb. Q stays in SBUF
  (no DRAM spill). Wq streamed per (tb, mg); x double-buffered.
- Softmax denominator: DVE bf16 accumulation of P tiles (2x mode) + one
  ones-stationary matmul per (head, block) — replaces 320 full-width PE
  matmuls with 32.
- Causal mask: narrow staircase matmuls in bf16 (moving width 128*(r+1))
  instead of full 512-wide f32r mask matmuls.
- exp over [128, 2*512] PSUM pairs (fewer Act instructions).
- Mixed precision: Q/K path f32r (scores sensitive), V/P/O/Wo/mask bf16.
- Phase C: wo resident bf16, stile on Act engine, one DMA per 128-row block.
"""
import numpy as np
import ml_dtypes

import concourse.bass as bass
import concourse.tile as tile
from concourse import bacc, mybir
from concourse.bass_utils import run_bass_kernel_spmd
from contextlib import ExitStack

F32 = mybir.dt.float32
F32R = mybir.dt.float32r
BF16 = mybir.dt.bfloat16
AF = mybir.ActivationFunctionType
AOp = mybir.AluOpType
BFNP = ml_dtypes.bfloat16

B, T, D = 4, 2048, 2048
H, Hkv = 16, 4
hD = 128
ROPE = 64
NB = T // 512          # 4 blocks of 512 tokens
HL = H // 2            # 8 heads per core
GL = Hkv // 2          # 2 kv heads per core

EXP_PAIR = True        # one exp over a 2-bank PSUM pair


def build_nc():
    nc = bacc.Bacc(trn_type="TRN2", target_bir_lowering=False, debug=False)
    xT = nc.dram_tensor("xT", [D, T], F32R, kind="ExternalInput").ap()
    wqT = nc.dram_tensor("wqT", [D, HL * hD], F32R, kind="ExternalInput").ap()
    wkT = nc.dram_tensor("wkT", [D, GL * hD], F32R, kind="ExternalInput").ap()
    wvT = nc.dram_tensor("wvT", [D, GL * hD], F32R, kind="ExternalInput").ap()
    woT = nc.dram_tensor("woT", [HL * hD, D], BF16, kind="ExternalInput").ap()
    cosb = nc.dram_tensor("cosb", [ROPE, T], F32R, kind="ExternalInput").ap()
    sinb = nc.dram_tensor("sinb", [ROPE, T], F32R, kind="ExternalInput").ap()
    maskb = nc.dram_tensor("maskb", [128, 4, 512], BF16, kind="ExternalInput").ap()
    ident = nc.dram_tensor("ident", [128, 128], BF16, kind="ExternalInput").ap()
    ones_c = nc.dram_tensor("ones_c", [128, 1], BF16, kind="ExternalInput").ap()
    ones_r = nc.dram_tensor("ones_r", [1, 128], F32R, kind="ExternalInput").ap()
    outT = nc.dram_tensor("outT", [D, T], F32, kind="ExternalOutput").ap()

    xTr = xT.rearrange("(n p) t -> p n t", p=128)      # [128, 16, 2048]
    wqTr = wqT.rearrange("(n p) m -> p n m", p=128)    # [128, 16, 1024]
    wkTr = wkT.rearrange("(n p) m -> p n m", p=128)    # [128, 16, 256]
    wvTr = wvT.rearrange("(n p) m -> p n m", p=128)
    woTr = woT.rearrange("(j p) m -> p j m", p=128)    # [128, 8, 2048]

    with tile.TileContext(nc) as tc, ExitStack() as ctx:
        const = ctx.enter_context(tc.tile_pool(name="const", bufs=1))
        persist = ctx.enter_context(tc.tile_pool(name="persist", bufs=1))
        qpool = ctx.enter_context(tc.tile_pool(name="qpool", bufs=1))
        psum = ctx.enter_context(tc.tile_pool(name="psum", bufs=1, space="PSUM"))
        work = ctx.enter_context(tc.tile_pool(name="work", bufs=2))
        ptp = ctx.enter_context(tc.tile_pool(name="ptp", bufs=3))
        accp = ctx.enter_context(tc.tile_pool(name="accp", bufs=2))

        # consts + resident weights on the Act queue (sync queue is for x/wq)
        tid = const.tile([128, 128], BF16, tag="tid")
        nc.scalar.dma_start(tid[:], ident)
        toc = const.tile([128, 1], BF16, tag="toc")
        nc.scalar.dma_start(toc[:], ones_c)
        tor = const.tile([1, 128], F32R, tag="tor")
        nc.scalar.dma_start(tor[:], ones_r)
        tmask = const.tile([128, 4, 512], BF16, tag="tmask")
        nc.scalar.dma_start(tmask[:], maskb)
        tcos = const.tile([ROPE, T], F32R, tag="tcos")
        nc.scalar.dma_start(tcos[:], cosb)
        tsin = const.tile([ROPE, T], F32R, tag="tsin")
        nc.scalar.dma_start(tsin[:], sinb)
        wkg = const.tile([128, 16, GL * hD], F32R, tag="wkg")
        nc.scalar.dma_start(wkg[:], wkTr)
        wvg = const.tile([128, 16, GL * hD], F32R, tag="wvg")
        nc.scalar.dma_start(wvg[:], wvTr)

        KT = persist.tile([128, GL, T], F32R, tag="KT")
        VT = persist.tile([128, T // 128, GL * hD], BF16, tag="VT")
        OT = persist.tile([128, HL, T], BF16, tag="OT")

        # streamed pools, released after the last stage to make room for wo
        xctx = ExitStack()
        xpool = xctx.enter_context(tc.tile_pool(name="xp", bufs=18))
        wqp = xctx.enter_context(tc.tile_pool(name="wqp", bufs=2))

        def rope_inplace(dst, sw_name):
            """dst: [128,512] slice (f32r); rotate channels 0:64 in place."""
            sw = work.tile([ROPE, 512], F32R, tag="sw", name=sw_name)
            nc.gpsimd.dma_start(sw[0:32, :], dst[32:64, :])
            nc.gpsimd.dma_start(sw[32:64, :], dst[0:32, :])
            ts_ = work.tile([ROPE, 512], F32R, tag="ts", name=sw_name + "s", bufs=1)
            tc_ = work.tile([ROPE, 512], F32R, tag="tc", name=sw_name + "c", bufs=1)
            nc.vector.tensor_mul(ts_[:], sw[:], tsin[:, tsl])
            nc.vector.tensor_mul(tc_[:], dst[0:ROPE, :], tcos[:, tsl])
            nc.vector.tensor_tensor(out=dst[0:ROPE, :], in0=tc_[:], in1=ts_[:], op=AOp.add)

        # x tiles for stage 0
        xtiles = {}
        for d in range(16):
            xt = xpool.tile([128, 512], F32R, tag="xt", name=f"xt0_{d}")
            nc.sync.dma_start(xt[:], xTr[:, d, 0:512])
            xtiles[(0, d)] = xt

        Qbs = {}
        for tb in range(NB):
            tsl = slice(512 * tb, 512 * (tb + 1))
            xts = [xtiles[(tb, d)] for d in range(16)]
            with nc.named_scope(f"A{tb}"):
                # ---- K projection + rope ----
                for g in range(GL):
                    pk = psum.tile([128, 512], F32, tag="pa", name=f"pk{tb}_{g}")
                    for d in range(16):
                        nc.tensor.matmul(pk[:], wkg[:][:, d, 128 * g:128 * (g + 1)],
                                         xts[d][:], start=(d == 0), stop=(d == 15))
                    nc.vector.tensor_copy(KT[:][:, g, tsl], pk[:])
                    rope_inplace(KT[:][:, g, tsl], f"ksw{tb}_{g}")
                # ---- V projection (natural layout, bf16) ----
                for tt in range(4):
                    tloc = 4 * tb + tt
                    pv = psum.tile([128, GL * hD], F32, tag="pa", name=f"pv{tb}_{tt}")
                    for d in range(16):
                        nc.tensor.matmul(pv[:], xts[d][:, 128 * tt:128 * (tt + 1)],
                                         wvg[:][:, d, :], start=(d == 0), stop=(d == 15))
                    nc.vector.tensor_copy(VT[:][:, tloc, :], pv[:])
                # ---- Q projection + rope (wq streamed per mg) ----
                Qb = qpool.tile([128, HL, 512], F32R, tag="qb", name=f"qb{tb}")
                Qbs[tb] = Qb
                for h in range(HL):
                    wqg = wqp.tile([128, 16, 128], F32R, tag="wqg", name=f"wqg{tb}_{h}")
                    nc.sync.dma_start(wqg[:], wqTr[:, :, 128 * h:128 * (h + 1)])
                    pq = psum.tile([128, 512], F32, tag="pa", name=f"pq{tb}_{h}")
                    for d in range(16):
                        nc.tensor.matmul(pq[:], wqg[:][:, d, :],
                                         xts[d][:], start=(d == 0), stop=(d == 15))
                    nc.vector.tensor_copy(Qb[:][:, h, :], pq[:])
                    rope_inplace(Qb[:][:, h, :], f"qsw{tb}_{h}")
                # prefetch x for next stage (sync queue, after this stage's wq)
                if tb + 1 < NB:
                    for d in range(16):
                        xt = xpool.tile([128, 512], F32R, tag="xt", name=f"xt{tb + 1}_{d}")
                        nc.sync.dma_start(xt[:], xTr[:, d, slice(512 * (tb + 1), 512 * (tb + 2))])
                        xtiles[(tb + 1, d)] = xt

            if tb == NB - 1:
                # last stage: release x/wq space, start loading wo for phase C
                xctx.close()
                woctx = ExitStack()
                wop = woctx.enter_context(tc.tile_pool(name="wop", bufs=1))
                stgp = woctx.enter_context(tc.tile_pool(name="stgp", bufs=2))
                wo_sb = wop.tile([128, HL, D], BF16, tag="wo")
                nc.scalar.dma_start(wo_sb[:], woTr)

            # ---- B(:, tb): attention for queries in block tb ----
            with nc.named_scope(f"B{tb}"):
                Qb = Qbs[tb]
                nj = 4 * (tb + 1)
                for h in range(HL):
                    g = h // (HL // GL)
                    qmv = Qb[:][:, h, :]
                    po = psum.tile([128, 512], F32, tag="po", name=f"po{tb}_{h}")
                    acc = None
                    for pj in range(nj // 2):
                        st = psum.tile([128, 2, 512], F32, tag="st", name=f"st{tb}_{h}_{pj}")
                        pt = ptp.tile([128, 2, 512], BF16, tag="pt", name=f"pt{tb}_{h}_{pj}")
                        for jj in range(2):
                            j = 2 * pj + jj
                            diag = j >= 4 * tb
                            nc.tensor.matmul(st[:, jj, :], KT[:][:, g, 128 * j:128 * (j + 1)],
                                             qmv, start=True, stop=not diag)
                            if diag:
                                r = j - 4 * tb
                                w = 128 * (r + 1)
                                nc.tensor.matmul(st[:, jj, 0:w], tid[:],
                                                 tmask[:][:, r, 0:w], start=False, stop=True)
                        if EXP_PAIR:
                            nc.scalar.activation(pt[:], st[:], AF.Exp)
                        else:
                            for jj in range(2):
                                nc.scalar.activation(pt[:, jj, :], st[:, jj, :], AF.Exp)
                        for jj in range(2):
                            j = 2 * pj + jj
                            nc.tensor.matmul(po[:], VT[:][:, j, 128 * g:128 * (g + 1)],
                                             pt[:, jj, :], start=(j == 0), stop=(j == nj - 1))
                        # denominator accumulation on DVE (bf16 2x)
                        if pj == 0:
                            acc = accp.tile([128, 512], BF16, tag="acc", name=f"ac{tb}_{h}_0")
                            nc.vector.tensor_tensor(out=acc[:], in0=pt[:, 0, :],
                                                    in1=pt[:, 1, :], op=AOp.add)
                        else:
                            for jj in range(2):
                                nacc = accp.tile([128, 512], BF16, tag="acc",
                                                 name=f"ac{tb}_{h}_{2 * pj + jj}")
                                nc.vector.tensor_tensor(out=nacc[:], in0=acc[:],
                                                        in1=pt[:, jj, :], op=AOp.add)
                                acc = nacc
                    # reduce over keys (partition) with a ones-stationary matmul
                    pdr = psum.tile([1, 512], F32, tag="pa", name=f"pdr{tb}_{h}")
                    nc.tensor.matmul(pdr[:], toc[:], acc[:], start=True, stop=True)
                    rsb = work.tile([1, 512], F32, tag="rsb", name=f"rsb{tb}_{h}", bufs=1)
                    nc.vector.reciprocal_approx_fast(rsb[:], pdr[:])
                    rsr = work.tile([1, 512], F32R, tag="rsr", name=f"rsr{tb}_{h}", bufs=1)
                    nc.vector.tensor_copy(rsr[:], rsb[:])
                    pr = psum.tile([128, 512], F32, tag="pa", name=f"pr{tb}_{h}")
                    nc.tensor.matmul(pr[:], tor[:], rsr[:], start=True, stop=True)
                    rps = work.tile([128, 512], F32, tag="rps", name=f"rps{tb}_{h}", bufs=1)
                    nc.vector.tensor_copy(rps[:], pr[:])
                    nc.vector.tensor_tensor(out=OT[:][:, h, tsl], in0=po[:], in1=rps[:],
                                            op=AOp.mult)

        # ---------------- phase C: output projection (transposed) ----------------
        with nc.named_scope("C"):
            for m2 in range(16):
                msl = slice(128 * m2, 128 * (m2 + 1))
                stile = stgp.tile([128, T], F32, tag="stile", name=f"stile{m2}")
                for tbl in range(NB):
                    pc = psum.tile([128, 512], F32, tag="pa", name=f"pc{m2}_{tbl}")
                    for j in range(HL):
                        nc.tensor.matmul(pc[:], wo_sb[:][:, j, msl],
                                         OT[:][:, j, 512 * tbl:512 * (tbl + 1)],
                                         start=(j == 0), stop=(j == HL - 1))
                    nc.scalar.activation(stile[:, 512 * tbl:512 * (tbl + 1)], pc[:], AF.Copy)
                nc.sync.dma_start(outT[msl, :], stile[:])
        woctx.close()
    nc.compile()
    return nc


# de-interleave permutation for rope channels: x1 (even) -> 0:32, x2 (odd) -> 32:64
_PERM = np.concatenate([np.arange(0, ROPE, 2), np.arange(1, ROPE, 2), np.arange(ROPE, hD)])


def prepare_inputs(x, cos, sin, Wq, Wkv, Wo, q_gain):
    """Host-side sharding + layout prep. Returns list of 8 in_maps."""
    x = np.asarray(x, np.float32)
    cos = np.asarray(cos, np.float32)
    sin = np.asarray(sin, np.float32)
    Wq = np.asarray(Wq, np.float32)
    Wkv = np.asarray(Wkv, np.float32)
    Wo = np.asarray(Wo, np.float32)
    q_gain = np.asarray(q_gain, np.float32)

    # rope tables in de-interleaved order: C = [cos; cos], S = [-sin; +sin]
    cosb = np.ascontiguousarray(np.concatenate([cos.T, cos.T], axis=0))   # [64, T]
    sinb = np.ascontiguousarray(np.concatenate([-sin.T, sin.T], axis=0))  # [64, T]

    # additive causal masks for diagonal s-tiles, r = j - 4*ib
    p = np.arange(128)[:, None]
    f = np.arange(512)[None, :]
    maskb = np.zeros((128, 4, 512), np.float32)
    for r in range(4):
        maskb[:, r, :] = np.where(p + 128 * r > f, -1e9, 0.0)

    ident = np.eye(128, dtype=np.float32)
    ones_c = np.ones((128, 1), np.float32)
    ones_r = np.ones((1, 128), np.float32)

    scale = 1.0 / np.sqrt(hD)
    xT = [np.ascontiguousarray(x[b].T) for b in range(B)]

    in_maps = []
    for c in range(8):
        b, hf = divmod(c, 2)
        heads = np.arange(hf * HL, (hf + 1) * HL)
        Wq_h = Wq.reshape(H, hD, D)[heads] * (q_gain[heads, None, None] * scale)
        Wq_h = Wq_h[:, _PERM, :]                                     # de-interleave rope chans
        kvh = np.arange(hf * GL, (hf + 1) * GL)
        Wkv_r = Wkv.reshape(Hkv, 2 * hD, D)[kvh]
        Wk_h = Wkv_r[:, :hD, :][:, _PERM, :]
        Wv_h = Wkv_r[:, hD:, :]
        Wo_h = Wo[:, hf * HL * hD:(hf + 1) * HL * hD]

        in_maps.append({
            "xT": xT[b],
            "wqT": np.ascontiguousarray(Wq_h.reshape(HL * hD, D).T),
            "wkT": np.ascontiguousarray(Wk_h.reshape(GL * hD, D).T),
            "wvT": np.ascontiguousarray(Wv_h.reshape(GL * hD, D).T),
            "woT": np.ascontiguousarray(Wo_h.T).astype(BFNP),
            "cosb": cosb, "sinb": sinb,
            "maskb": maskb.astype(BFNP),
            "ident": ident.astype(BFNP),
            "ones_c": ones_c.astype(BFNP),
            "ones_r": ones_r,
        })
    return in_maps


_NC_CACHE = {}


def kernel(x, cos, sin, Wq, Wkv, Wo, q_gain, _trace=False):
    if "nc" not in _NC_CACHE:
        _NC_CACHE["nc"] = build_nc()
    nc = _NC_CACHE["nc"]
    in_maps = prepare_inputs(x, cos, sin, Wq, Wkv, Wo, q_gain)
    res = run_bass_kernel_spmd(nc, in_maps, core_ids=list(range(8)), trace=_trace)
    if _trace:
        _NC_CACHE["last_results"] = res
    out = np.empty((B, T, D), np.float32)
    for b in range(B):
        acc = res.results[2 * b]["outT"] + res.results[2 * b + 1]["outT"]
        out[b] = acc.T
    return out
